# revision 8
# baseline (speedup 1.0000x reference)
"""Trainium2 Bass kernel for the dense real-space long-range kernel
(N=6144 atoms, B=8 periodic cells, screened-Coulomb pair energy with
minimum-image convention, row-summed per atom).

batch is sorted and cross-graph pairs are masked, so the N x N problem is
block-diagonal over the 8 graphs; one graph per NeuronCore.  Pair math in
fractional coordinates; work is split into (macro row-block, column-chunk)
units, upper-block-triangular (block-row m covers cols >= 123*m).

Per unit (3 groups of 41 atoms, rows (i,k) k-interleaved):
  groups 0,1 (A-form, aux-row trick):
    f   = fb + negf                      (Pool broadcast-add)
    r   = (f + MAGIC) - MAGIC = round(f) (DVE, exact ints into f32r rw tile)
    y   = WAUX^T @ [r; fb3]              (PE)  = C.fb - C.r   [no bias]
    sq  = Square(y + bias)               (ACT, bias = -(frac_i @ C), f16)
  group 2 (B-form):
    f   = fb + negf                      (DVE ts_ptr)
    r   = magic round                    (DVE)
    w   = f - r                          (Pool tt)
    v   = GBS^T @ w                      (PE, Gram = C C^T)
    sq  = w * v                          (DVE stt, one PSUM operand, f16)
  q   = onesb^T @ sq  (PE, PSUM accumulate over 3 groups)
  l   = ln(q + soft^2)              (ACT Ln)
  z   = exp(0.5 l + ln sigma)       (ACT Exp)  [= sigma*r]
  n   = z + 0.5 l                   (DVE stt)
  kern= exp(-n) = exp(-sigma r)/r   (ACT Exp)
  acc[j] += src_m^T @ kern (PE matvec into pre-zeroed PSUM, start=False)
  ra[i]   = sum_j kern*src_j over cols right of the diagonal block (DVE stt)
E_i = 0.5*src_i*(acc_i + ra_i) - 0.5*src_i^2*exp(-sigma*soft)/soft  (host)

Units are software-pipelined with LAG; sq2/q2 of unit i are emitted with
stage1 of unit i+1 to avoid head-of-line stalls on the w->v->sq2 chain.
"""
import numpy as np

GA = 40            # atoms per k-interleaved row group
ROWS = 3 * GA      # 120 partitions of pair rows per group tile
AUX = 4            # aux rows: 3 fb base rows + 1 zero pad (even fp32r geometry)
GPM = 3            # groups per macro block
MACRO = GA * GPM   # 123 atoms per macro
MAGIC = 12582912.0  # 1.5 * 2**23: (x + MAGIC) - MAGIC == round(x) for |x| < 2**22
NCORES = 8
CHUNK = 512        # PSUM bank / fp32 matmul free-dim limit
MINW = 128
LAG = 2
NPHASE = 3         # rw-tile phases (pipeline depth for full-width moving tiles)

_cache = {}


def _units(n_macros, cols):
    """(m, ca, cb, mva): compute cols [ca,cb), matvec/reduce from mva."""
    units = []
    for m in range(n_macros):
        c0 = MACRO * m
        pts = ([c0]
               + [p for p in range(((c0 // CHUNK) + 1) * CHUNK, cols, CHUNK)]
               + [cols])
        segs = [[pts[i], pts[i + 1]] for i in range(len(pts) - 1)]
        i = 0
        while i < len(segs) - 1:
            if (segs[i][1] - segs[i][0] < MINW
                    or segs[i + 1][1] - segs[i + 1][0] < MINW) \
                    and segs[i + 1][1] - segs[i][0] <= CHUNK:
                segs[i] = [segs[i][0], segs[i + 1][1]]
                del segs[i + 1]
            else:
                i += 1
        for a, b in segs:
            ca = a if b - a >= MINW else max(0, b - MINW)
            units.append((m, ca, b, a))
    return units


def _mv_pieces(mva, cb_):
    """Split [mva, cb_) at CHUNK-grid (PSUM bank) boundaries."""
    pieces = []
    p = mva
    while p < cb_:
        pn = min(cb_, (p // CHUNK + 1) * CHUNK)
        pieces.append((p, pn))
        p = pn
    return pieces


def _build(n_macros, cols, sigma, soft):
    import concourse.bacc as bacc
    import concourse.mybir as mybir
    import concourse.tile as tile

    f32 = mybir.dt.float32
    f32r = mybir.dt.float32r
    f16 = mybir.dt.float16
    alu = mybir.AluOpType
    act = mybir.ActivationFunctionType

    n_groups = GPM * n_macros
    soft2 = float(np.float32(soft) * np.float32(soft))
    lnsig = float(np.log(np.float64(sigma)))
    units = _units(n_macros, cols)
    nu = len(units)
    KR = ROWS + AUX  # 126

    nc = bacc.Bacc("TRN2", target_bir_lowering=False, debug=False)

    for name, val in [("soft2", soft2), ("lnsig", lnsig)]:
        t = nc.alloc_sbuf_tensor(f"const-{name}", [128, 1], f32)
        nc.gpsimd.memset(t.ap(), val)
        nc.const_aps.aps[(f32, val)] = t.ap()
    nc.all_engine_barrier()
    # pin the ACT table serving Ln+Exp+Square (set 6) -> no table reloads
    nc.scalar.add_instruction(mybir.InstLoadActFuncSet(
        name=nc.get_next_instruction_name(), act_func_set_id=6, ins=[], outs=[]))

    # consts packed into few params: DMA cost here is per-partition
    # descriptor count, not bytes, so small tensors ride together
    FBR = nc.declare_dram_parameter("FBR", [KR, cols], f32r, isOutput=False)
    WG = nc.declare_dram_parameter("WG", [KR, ROWS], f32r, isOutput=False)
    SMALL = nc.declare_dram_parameter("SMALL", [ROWS, 2 * n_groups + n_macros],
                                      f32, isOutput=False)
    SRCO = nc.declare_dram_parameter("SRCO", [ROWS, GPM * MACRO + cols], f16,
                                     isOutput=False)
    OUTA = nc.declare_dram_parameter("OUTA", [1, cols], f32, isOutput=True)
    OUTR = nc.declare_dram_parameter("OUTR", [MACRO, nu], f32, isOutput=True)

    with tile.TileContext(nc) as tc:
        with tc.tile_pool(name="const", bufs=1) as cpool, \
             tc.tile_pool(name="work", bufs=3) as pool, \
             tc.tile_pool(name="ypsum", bufs=3, space="PSUM") as ypool, \
             tc.tile_pool(name="qpsum", bufs=3, space="PSUM") as qpool, \
             tc.tile_pool(name="apsum", bufs=1, space="PSUM") as apool:
            fbr = cpool.tile([KR, cols], f32r)
            wg = cpool.tile([KR, ROWS], f32r)
            small = cpool.tile([ROWS, 2 * n_groups + n_macros], f32)
            srco = cpool.tile([ROWS, GPM * MACRO + cols], f16)
            srcst = cpool.tile([MACRO, n_macros], f16)
            ra = cpool.tile([MACRO, nu], f32)
            # spread const loads across the three DMA-capable rings
            nc.sync.dma_start(small[:], SMALL[:])
            nc.sync.dma_start(fbr[:], FBR[:])
            nc.scalar.dma_start(wg[:], WG[:])
            nc.scalar.dma_start(srco[:], SRCO[:])
            nc.gpsimd.memset(ra[:], 0.0)
            waux = wg[:, 0:ROWS]
            negf = small[:, 0:n_groups]
            biasp = small[:, n_groups:2 * n_groups]
            onesb = srco[:, 0:GPM * MACRO]
            srcb = srco[:, GPM * MACRO:]
            nc.vector.tensor_copy(
                srcst[:], small[:, 2 * n_groups:2 * n_groups + n_macros])

            # full-width A-form moving tiles; aux rows = fb base rows (+zero
            # pad row), DMAed once
            rw = [[cpool.tile([KR, cols], f32r, name=f"rw{g}_{p}")
                   for p in range(NPHASE)] for g in range(GPM)]
            for g in range(GPM):
                for p in range(NPHASE):
                    eng = (nc.sync, nc.scalar, nc.gpsimd)[p]
                    eng.dma_start(rw[g][p][ROWS:KR, :], FBR[ROWS:KR, :])

            acc = apool.tile([1, -(-cols // CHUNK) * CHUNK], f32)
            nc.vector.memset(acc[:], 0.0)

            qmeta = {}

            def stage1(ui):
                m, ca, cb_, mva = units[ui]
                cs = cb_ - ca
                ph = ui % NPHASE
                q = qpool.tile([MACRO, CHUNK], f32, tag="q")
                ys = []
                for tgi in range(GPM):
                    g = GPM * m + tgi
                    fa = pool.tile([ROWS, CHUNK], f32, tag=f"fa{tgi}")
                    nc.gpsimd.tensor_tensor(
                        fa[:, 0:cs], fbr[0:ROWS, ca:cb_],
                        negf[:, g:g + 1].to_broadcast([ROWS, cs]), alu.add)
                    rt = rw[tgi][ph]
                    nc.vector.tensor_scalar(
                        rt[0:ROWS, ca:cb_], fa[:, 0:cs], MAGIC, MAGIC,
                        alu.add, alu.subtract)
                    y = ypool.tile([MACRO, CHUNK], f32, tag="y")
                    nc.tensor.matmul(y[:, 0:cs], waux, rt[:, ca:cb_],
                                     start=True, stop=True)
                    ys.append((g, y))
                for tgi, (g, y) in enumerate(ys):
                    sq = pool.tile([ROWS, CHUNK], f16, tag=f"sq{tgi}")
                    nc.scalar.activation(sq[:, 0:cs], y[:, 0:cs], act.Square,
                                         bias=biasp[:, g:g + 1])
                    nc.tensor.matmul(q[:, 0:cs],
                                     onesb[:, MACRO * tgi:MACRO * (tgi + 1)],
                                     sq[:, 0:cs], start=(tgi == 0),
                                     stop=(tgi == GPM - 1))
                qmeta[ui] = q

            kmeta = {}

            def stage2a(ui):
                m, ca, cb_, mva = units[ui]
                cs = cb_ - ca
                q = qmeta.pop(ui)
                l = pool.tile([MACRO, CHUNK], f16, tag="l")
                nc.scalar.activation(l[:, 0:cs], q[:, 0:cs], act.Ln, bias=soft2)
                z = pool.tile([MACRO, CHUNK], f16, tag="z")
                nc.scalar.activation(z[:, 0:cs], l[:, 0:cs], act.Exp,
                                     bias=lnsig, scale=0.5)
                n = pool.tile([MACRO, CHUNK], f16, tag="n")
                nc.vector.scalar_tensor_tensor(n[:, 0:cs], l[:, 0:cs], 0.5,
                                               z[:, 0:cs], alu.mult, alu.add)
                kern = pool.tile([MACRO, CHUNK], f16, tag="kern")
                nc.scalar.activation(kern[:, 0:cs], n[:, 0:cs], act.Exp,
                                     scale=-1.0)
                kmeta[ui] = kern

            def stage2b(ui):
                m, ca, cb_, mva = units[ui]
                cs = cb_ - ca
                kern = kmeta.pop(ui)
                pieces = _mv_pieces(mva, cb_)
                for pi, (p0, p1) in enumerate(pieces):
                    nc.tensor.matmul(acc[0:1, p0:p1], srcst[:, m:m + 1],
                                     kern[:, p0 - ca:p1 - ca], start=False,
                                     stop=(ui == nu - 1 and pi == len(pieces) - 1),
                                     skip_group_check=True)
                lo = max(MACRO * (m + 1), mva)
                if lo < cb_:
                    kw = pool.tile([MACRO, CHUNK], f16, tag="kw")
                    nc.vector.scalar_tensor_tensor(
                        kw[:, 0:cb_ - lo], kern[:, lo - ca:cs], 1.0,
                        srcb[:, lo:cb_], alu.mult, alu.mult,
                        accum_out=ra[:, ui:ui + 1])

            # stage2a first each iteration (its inputs are long ready;
            # emitting it ahead avoids queueing behind stage1 ops that wait on
            # cross-engine chains); mv/kw go last so they don't block PE/DVE
            # queue heads on the kern chain
            for i in range(nu + LAG):
                if i >= LAG:
                    stage2a(i - LAG)
                if i < nu:
                    stage1(i)
                if i >= LAG:
                    stage2b(i - LAG)

            eo = pool.tile([1, cols], f32, tag="eo")
            nc.vector.tensor_scalar(eo[:], acc[0:1, 0:cols], 1.0, None, alu.mult)
            nc.sync.dma_start(OUTA[:], eo[:])
            nc.sync.dma_start(OUTR[:], ra[:])
    nc.compile()
    return nc


def _get_program(n_macros, cols, sigma, soft):
    key = (n_macros, cols, round(sigma, 9), round(soft, 9))
    if key not in _cache:
        _cache[key] = _build(n_macros, cols, sigma, soft)
    return _cache[key]


LAST_EXEC_TIME_NS = None


def kernel(pos, batch, cell, source, screening, softening, *, _trace=False):
    global LAST_EXEC_TIME_NS
    from concourse.bass_utils import run_bass_kernel_spmd

    pos = np.asarray(pos)
    batch = np.asarray(batch)
    cell = np.asarray(cell)
    source = np.asarray(source, dtype=np.float32)
    sigma = float(np.asarray(screening, dtype=np.float32))
    soft = float(np.asarray(softening, dtype=np.float32))

    n = pos.shape[0]
    nb = cell.shape[0]
    bi = batch.astype(np.int64)
    counts = np.bincount(bi, minlength=nb)
    starts = np.concatenate([[0], np.cumsum(counts)])
    assert nb == NCORES and np.all(np.diff(bi) >= 0)

    # host precompute in float64
    inv = np.linalg.inv(cell.astype(np.float64))
    frac = np.empty((n, 3), dtype=np.float64)
    for g in range(nb):
        i0, i1 = starts[g], starts[g + 1]
        frac[i0:i1] = pos[i0:i1].astype(np.float64) @ inv[g]

    namax = int(counts.max())
    n_macros = -(-namax // MACRO)
    cols = namax + (namax % 2)    # columns trimmed to real max atoms (even)
    rows_tot = MACRO * n_macros   # row padding to full macros
    n_groups = GPM * n_macros
    diag_c = float(np.exp(-np.float64(sigma) * np.float64(soft)) / np.float64(soft))
    units = _units(n_macros, cols)
    nu = len(units)
    KR = ROWS + AUX

    idx_atom = np.arange(ROWS) // 3
    idx_k = np.arange(ROWS) % 3

    in_maps = []
    spads = []
    for g in range(nb):
        i0, i1 = starts[g], starts[g + 1]
        ng = i1 - i0
        fpad = np.zeros((rows_tot, 3), dtype=np.float64)
        fpad[:ng] = frac[i0:i1]
        fpad32 = fpad.astype(np.float32)
        spad = np.zeros(rows_tot, dtype=np.float32)
        spad[:ng] = source[i0:i1]
        spads.append(spad)

        fbr = np.zeros((KR, cols), dtype=np.float32)
        fbr[:ROWS] = np.tile(fpad32[:cols].T, (GA, 1))
        fbr[ROWS:ROWS + 3] = fpad32[:cols].T
        fbv = np.ascontiguousarray(np.tile(fpad32[:cols].T, (GA, 1)))      # [123, cols]
        negfa = np.zeros((ROWS, n_groups), dtype=np.float32)
        biasp = np.zeros((ROWS, n_groups), dtype=np.float32)
        C = cell[g].astype(np.float64)
        G = (C @ C.T)
        pseudo = fpad @ C            # ~pos of each padded atom, f64
        for t in range(n_groups):
            a = t * GA + idx_atom
            negfa[:, t] = -fpad32[a, idx_k]
            biasp[:, t] = (-pseudo[a, idx_k]).astype(np.float32)
        waux = np.zeros((KR, ROWS), dtype=np.float32)
        C32 = C.astype(np.float32)
        for i in range(GA):
            waux[3 * i:3 * i + 3, 3 * i:3 * i + 3] = -C32
            waux[ROWS:ROWS + 3, 3 * i:3 * i + 3] = C32
        onesb = np.zeros((ROWS, GPM, MACRO), dtype=np.float32)
        for t in range(GPM):
            for i in range(GA):
                onesb[3 * i:3 * i + 3, t, GA * t + i] = 1.0
        onesb = np.ascontiguousarray(onesb.reshape(ROWS, GPM * MACRO)).astype(np.float16)
        srcst = np.zeros((MACRO, n_macros), dtype=np.float32)
        for m in range(n_macros):
            srcst[:, m] = spad[m * MACRO: m * MACRO + MACRO]
        srcb = np.ascontiguousarray(np.tile(spad[None, :cols], (MACRO, 1))).astype(np.float16)
        wgp = waux
        small = np.concatenate(
            [negfa, biasp, srcst.astype(np.float32)], axis=1)
        srco = np.concatenate([onesb, srcb], axis=1)
        in_maps.append({"FBR": fbr, "WG": wgp, "SMALL": small, "SRCO": srco})

    nc = _get_program(n_macros, cols, sigma, soft)
    res = run_bass_kernel_spmd(nc, in_maps, list(range(NCORES)), trace=_trace)
    LAST_EXEC_TIME_NS = res.exec_time_ns

    out = np.zeros((n, 1), dtype=np.float32)
    for g in range(nb):
        i0, i1 = starts[g], starts[g + 1]
        ng = i1 - i0
        acc = res.results[g]["OUTA"][0].astype(np.float64)   # [cols]
        rag = res.results[g]["OUTR"].astype(np.float64)      # [123, nu]
        rsum = np.zeros(MACRO * n_macros, dtype=np.float64)
        for ui, (m, ca, cb_, mva) in enumerate(units):
            lo = max(MACRO * (m + 1), mva)
            if lo < cb_:
                rsum[m * MACRO:(m + 1) * MACRO] += rag[:, ui]
        tot = np.concatenate([acc, np.zeros(MACRO * n_macros - cols)]) + rsum
        spad = spads[g].astype(np.float64)
        e = 0.5 * spad * tot - 0.5 * spad * spad * diag_c
        out[i0:i1, 0] = e[:ng].astype(np.float32)
    return out
